# revision 69
# baseline (speedup 1.0000x reference)
"""Slot-attention (softmax over queries + key renormalization) on 8 TRN2 NeuronCores.

Sharding: data-parallel over batch (b=8 -> one batch element per core, no
collectives). Per-core fused kernel, built around:

1. All inputs arrive via gpsimd SWDGE casting DMAs (f32 DRAM -> bf16 SBUF:
   half the serialized DMA-engine hold of an f32 load, no convert pass).
   All layout transposes run on the PE with a bf16 identity in [P,1024]
   single-bank bf16 psum groups (a p-state warmup keeps the PE at full
   clock through the prefix). Mixing HWDGE DMAs into this stream is
   poisonous: the rotating 8-deep DMA semaphore rings chain unrelated
   DMAs together; keep the HWDGE queue for the output stores only.

2. attn@v runs in the [i, e] orientation with the exp tiles as the
   STATIONARY operand: av[i-tile, 64] += expT[jt][:, i-slice]^T @ vhs[jt],
   full 128-row contraction (vs 64 in the [e, i] orientation) -> half the
   PE cycles. A 1-column matmul per (it, jt) accumulates the renormalizer
   r[i] into a persistent PSUM tile. Each head's av psum bank is released
   by a single copy to SBUF; normalization attn*[1/r[i]] is a
   per-partition scalar multiply on Pool from that copy, and the
   normalized [i, e] tiles are transposed back to [e, i] on the PE.

Pipeline per head h, j-tile jt: sim (PE) -> exp (ACT; S[j] via the ACT
accumulator on even tiles, via a DVE reduce on odd tiles to shorten the
serial ACT stream) -> [AVLAG steps later] invS column recip (DVE) + vhs
(Pool) + 16 av/r matmuls (PE). Prep (weight/context streams, v/q/k
projections, closed pairs' output projections) is interleaved as
budget-drained filler generators; the tail splits the last pair's
normalize/transpose/evacuate chains across DVE and ACT in parallel.
The end-to-end critical path is the serial exp stream on ACT.

Matmul operands bf16, accumulation f32 in PSUM, softmax statistics f32.
"""

import os
import sys

sys.path.insert(0, "/opt/trn_rl_repo")

import numpy as np

import concourse.bass as bass
import concourse.mybir as mybir
import concourse.tile as tile
from concourse import bacc
from concourse.bass_utils import run_bass_kernel_spmd
from concourse.masks import make_identity

B = 8
N = 1024  # queries
M = 1024  # keys
D = 512   # model dim
H = 8
DH = 64
INNER = H * DH
SCALE = DH ** -0.5
P = 128

F32 = mybir.dt.float32
CDT = mybir.dt.bfloat16

AV0LAG = int(os.environ.get("AV0LAG", "6"))
AVPACE = int(os.environ.get("AVPACE", "3"))
AVLAG = int(os.environ.get("AVLAG", "2"))
FB = int(os.environ.get("FB", "3"))
_DRAINED = {}

Exp = mybir.ActivationFunctionType.Exp
Mult = mybir.AluOpType.mult
Add = mybir.AluOpType.add


def _r3(ap, a):
    return ap.rearrange("p (a b) -> p a b", a=a)


def build(nc: bass.Bass):
    _DRAINED.clear()
    x_d = nc.declare_dram_parameter("x", [N, D], F32, isOutput=False)
    c_d = nc.declare_dram_parameter("context", [M, D], F32, isOutput=False)
    wq_d = nc.declare_dram_parameter("Wq", [INNER, D], F32, isOutput=False)
    wk_d = nc.declare_dram_parameter("Wk", [INNER, D], F32, isOutput=False)
    wv_d = nc.declare_dram_parameter("Wv", [INNER, D], F32, isOutput=False)
    wo_d = nc.declare_dram_parameter("Wo", [D, INNER], F32, isOutput=False)
    bo_d = nc.declare_dram_parameter("bo", [D], F32, isOutput=False)
    out_d = nc.declare_dram_parameter("out", [N, D], F32, isOutput=True)

    with tile.TileContext(nc) as tc:
        with tc.tile_pool(name="const", bufs=1) as const:
            ident = const.tile([P, P], F32, tag="ident")
            ident_b = const.tile([P, P], CDT, tag="ident_b")
            ones128 = const.tile([1, P], CDT, tag="ones128")
            nc.gpsimd.memset(ones128[:, :], 1.0)
            # tiny warmup exp so the ACT table load happens at t~0
            warm = const.tile([1, 2], F32, tag="warm")
            nc.gpsimd.memset(warm[:, :], 0.0)
            nc.scalar.activation(warm[:, 0:1], warm[:, 1:2], Exp, scale=1.0)
            bo_s = const.tile([1, D], F32, tag="bo_s")
            bo_sb = const.tile([1, D], CDT, tag="bo_sb")
            bo_b = const.tile([P, D], F32, tag="bo_b")

            with tc.tile_pool(name="stage", bufs=1) as stage:
                # wT[n][p, (et*4+dt)*128 + f] = W[et*128+f, dt*128+p]
                wT = {n: stage.tile([P, 16 * P], CDT, tag=f"w{n}T", name=f"w{n}T")
                      for n in ("q", "k", "v")}
                # woT[p, (dt*4+et)*128 + f] = Wo[dt*128+f, et*128+p]
                woT = stage.tile([P, 16 * P], CDT, tag="woT")
                # xT_b[p, (nt*4+dt)*128 + f] = x[nt*128+f, dt*128+p]
                xT = stage.tile([P, 32 * P], CDT, tag="xT")
                cT = stage.tile([P, 32 * P], CDT, tag="cT")
                v = [stage.tile([P, INNER], CDT, tag=f"v{t}", name=f"v{t}")
                     for t in range(8)]

                with tc.tile_pool(name="outp", bufs=1) as outp:
                    qT = [outp.tile([P, N], CDT, tag=f"qT{t}", name=f"qT{t}") for t in range(4)]
                    kT = [outp.tile([P, M], CDT, tag=f"kT{t}", name=f"kT{t}") for t in range(4)]
                    outT = [outp.tile([P, N], CDT, tag=f"outT{t}", name=f"outT{t}") for t in range(4)]
                    y_acc = [outp.tile([P, D], CDT, tag=f"y_acc{t}", name=f"y_acc{t}")
                             for t in range(8)]

                    with tc.tile_pool(name="head", bufs=1) as head, \
                         tc.tile_pool(name="norm", bufs=1) as norm, \
                         tc.tile_pool(name="ld", bufs=1) as ld:
                        ps_at = tc.alloc_tile_pool(name="ps_at", bufs=1, space="PSUM")
                        ps_rp = tc.alloc_tile_pool(name="ps_rp", bufs=1, space="PSUM")
                        r_all = ps_rp.tile([P, 64], F32, tag="r_all")
                        st = _State(nc, tc, head, norm, ld, ps_at, r_all,
                                    wT, woT, xT, cT, v, qT, kT, outT, y_acc,
                                    ident_b, ones128, bo_s, bo_sb, bo_b,
                                    x_d, c_d, wq_d, wk_d, wv_d, wo_d, bo_d, out_d)
                        st.ident = ident
                        _run(st)
                        ps_rp.release()
                        ps_at.release()
    return nc


class _State:
    def __init__(self, nc, tc, head, norm, ld, ps_at, r_all,
                 wT, woT, xT, cT, v, qT, kT, outT, y_acc,
                 ident_b, ones128, bo_s, bo_sb, bo_b,
                 x_d, c_d, wq_d, wk_d, wv_d, wo_d, bo_d, out_d):
        self.nc = nc
        self.tc = tc
        self.head = head
        self.norm = norm
        self.ld = ld
        self.ps = ps_at
        self.r_all = r_all
        self.wT = wT
        self.woT = woT
        self.xT = xT
        self.cT = cT
        self.v = v
        self.qT = qT
        self.kT = kT
        self.outT = outT
        self.y_acc = y_acc
        self.ident_b = ident_b
        self.ones128 = ones128
        self.bo_s = bo_s
        self.bo_sb = bo_sb
        self.bo_b = bo_b
        self.x_d = x_d
        self.c_d = c_d
        self.w_d = {"q": wq_d, "k": wk_d, "v": wv_d}
        self.wo_d = wo_d
        self.bo_d = bo_d
        self.out_d = out_d
        self.fillers = []
        self.invr = {}
        self.avn = {}


def _drain(g):
    if g is not None:
        for _ in g:
            pass


def _budget_drain(st, budget):
    while budget > 0 and st.fillers:
        try:
            next(st.fillers[0][1])
            _DRAINED[st.fillers[0][0]] = _DRAINED.get(st.fillers[0][0], 0) + 1
            budget -= 1
        except StopIteration:
            st.fillers.pop(0)


def _force(st, name):
    for pair in list(st.fillers):
        if pair[0] == name:
            _drain(pair[1])
            st.fillers.remove(pair)


def _force_until(st, name, count):
    while st.fillers and st.fillers[0][0] == name and \
            _DRAINED.get(name, 0) < count:
        try:
            next(st.fillers[0][1])
            _DRAINED[name] = _DRAINED.get(name, 0) + 1
        except StopIteration:
            st.fillers.pop(0)


# ---------------------------------------------------------------- prep ----

def _xbar(nc, dst3, src2):
    nc.sync.dma_start(dst3, src2, transpose=True)


def _cast_load(st, name, dram_ap, cols, tag, bufs):
    sb = st.ld.tile([P, cols], CDT, tag=tag, bufs=bufs, name=name)
    st.nc.gpsimd.dma_start(sb[:, :], dram_ap)
    return sb


def _pe_xpose_group(st, dstT, dst_off, src, src_off, name, tag="ps_misc",
                    eng=None):
    """PE-transpose four [128,128] bf16 chunks into dstT[:, dst_off:+512]."""
    nc = st.nc
    tp = st.ps.tile([P, 512], CDT, tag=tag, bufs=2, name=name)
    for i in range(4):
        nc.tensor.transpose(tp[:, i * P:(i + 1) * P],
                            src[:, src_off + i * P:src_off + (i + 1) * P],
                            st.ident_b[:, :])
    eng = eng or nc.vector
    if hasattr(eng, "tensor_copy"):
        eng.tensor_copy(dstT[:, dst_off:dst_off + 512], tp[:, :])
    else:
        eng.copy(dstT[:, dst_off:dst_off + 512], tp[:, :])


def _pe_xpose_group2(st, dstT, dst_off, src, src_off, name, tag="ps_misc",
                     eng=None):
    """PE-transpose eight [128,128] bf16 chunks (two source tiles) into
    dstT[:, dst_off:+1024] via one single-bank [P,1024] bf16 psum tile."""
    nc = st.nc
    tp = st.ps.tile([P, 1024], CDT, tag=tag, bufs=2, name=name)
    for i in range(8):
        nc.tensor.transpose(tp[:, i * P:(i + 1) * P],
                            src[:, src_off + i * P:src_off + (i + 1) * P],
                            st.ident_b[:, :])
    _copy(eng or nc.vector, dstT[:, dst_off:dst_off + 1024], tp[:, :])


def _pe_xpose_tile(st, which, nt, src, col_off, tag="ps_misc", eng=None):
    """PE-transpose one [128, 512] source tile into the blocked xT_b/cT_b."""
    dstT = st.xT if which == "x" else st.cT
    _pe_xpose_group(st, dstT, nt * 512, src, col_off, f"tp{which}{nt}",
                    tag=tag, eng=eng)


def _copy(eng, dst, src):
    if hasattr(eng, "tensor_copy"):
        eng.tensor_copy(dst, src)
    else:
        eng.copy(dst, src)


def _q_proj(st, et, ic, tag="ps_misc", eng=None):
    nc = st.nc
    pp = st.ps.tile([P, 512], F32, tag=tag, bufs=2, name=f"pq{et}_{ic}")
    xr = st.xT[:, :].rearrange("p (nt dt f) -> p dt nt f", nt=8, dt=4)
    for dt in range(4):
        nc.tensor.matmul(
            pp[:, :],
            st.wT["q"][:, (et * 4 + dt) * P:(et * 4 + dt + 1) * P],
            xr[:, dt, 4 * ic:4 * ic + 4, :],
            start=(dt == 0), stop=(dt == 3))
        yield
    _copy(eng or nc.vector, st.qT[et][:, ic * 512:(ic + 1) * 512], pp[:, :])


def _k_proj_chunk(st, et, k, tag="ps_misc", eng=None):
    nc = st.nc
    pk = st.ps.tile([P, 256], F32, tag=tag, bufs=2, name=f"pk{et}_{k}")
    cr = st.cT[:, :].rearrange("p (nt dt f) -> p dt nt f", nt=8, dt=4)
    for dt in range(4):
        nc.tensor.matmul(
            pk[:, :],
            st.wT["k"][:, (et * 4 + dt) * P:(et * 4 + dt + 1) * P],
            cr[:, dt, 2 * k:2 * k + 2, :],
            start=(dt == 0), stop=(dt == 3))
        yield
    _copy(eng or nc.vector, st.kT[et][:, k * 256:(k + 1) * 256], pk[:, :])


def _k_proj(st, et, ic):
    nc = st.nc
    pk = st.ps.tile([P, 512], F32, tag="ps_misc", bufs=2, name=f"pkf{et}_{ic}")
    cr = st.cT[:, :].rearrange("p (nt dt f) -> p dt nt f", nt=8, dt=4)
    for dt in range(4):
        nc.tensor.matmul(
            pk[:, :],
            st.wT["k"][:, (et * 4 + dt) * P:(et * 4 + dt + 1) * P],
            cr[:, dt, 4 * ic:4 * ic + 4, :],
            start=(dt == 0), stop=(dt == 3))
        yield
    nc.vector.tensor_copy(st.kT[et][:, ic * 512:(ic + 1) * 512], pk[:, :])


def _g_cstream(st):
    """Context tiles 2..7: PE-transpose + kT[0] chunks per pair."""
    for k in range(1, 4):
        src = st.c1_sb if k == 1 else st.c23_sb
        src_off = 0 if k == 1 else (k - 2) * 1024
        _pe_xpose_group2(st, st.cT, 2 * k * 512, src, src_off, f"tpc{k}")
        yield
        yield
        yield from _k_proj_chunk(st, 0, k)
        yield


def _g_wv(st):
    for g in range(2):
        _pe_xpose_group2(st, st.wT["v"], g * 1024, st.wv_sb, g * 1024,
                         f"tpwv{g}")
        yield
        yield


def _g_vproj(st):
    nc = st.nc
    wr = st.wT["v"][:, :].rearrange("p (et dt f) -> p dt et f", et=4, dt=4)
    for mt in range(8):
        pv = st.ps.tile([P, INNER], F32, tag="ps_misc", bufs=2, name=f"pv{mt}")
        for dt in range(4):
            nc.tensor.matmul(
                pv[:, :],
                st.cT[:, (mt * 4 + dt) * P:(mt * 4 + dt + 1) * P],
                wr[:, dt, :, :],
                start=(dt == 0), stop=(dt == 3))
            if dt == 3:
                nc.vector.tensor_copy(st.v[mt][:, :], pv[:, :])
            yield


def _g_wrest(st):
    """Wq/Wk e-tiles 2..3 (0..1 were transposed in the prefix)."""
    for name in ("q", "k"):
        _pe_xpose_group2(st, st.wT[name], 1024, st.w23_sb[name], 0,
                         f"tpw{name}23")
        yield
        yield
        yield


def _g_proj(st, et):
    for ic in range(2):
        yield from _q_proj(st, et, ic)
        yield
        yield from _k_proj(st, et, ic)
        yield


def _g_wo(st):
    nc = st.nc
    sb = st.wo_sb
    for g in range(2):
        _pe_xpose_group2(st, st.woT, g * 1024, sb, g * 1024, f"tpwo{g}")
        yield
        yield
    nc.vector.tensor_copy(st.bo_sb[:, :], st.bo_s[:, :])
    pbo = st.ps.tile([P, D], F32, tag="ps_misc", bufs=2, name="pbo")
    nc.tensor.matmul(pbo[:, :], st.ones128[:, :], st.bo_sb[:, :],
                     start=True, stop=True)
    nc.vector.tensor_copy(st.bo_b[:, :], pbo[:, :])
    yield


def _g_ypair(st, et2, to_dram):
    nc = st.nc
    wr = st.woT[:, :].rearrange("p (dt et f) -> p et dt f", dt=4, et=4)
    for nt in range(8):
        tag = "ps_sim" if to_dram else "ps_misc"
        py = st.ps.tile([P, D], F32, tag=tag,
                        bufs=2, name=f"py{et2}_{nt}")
        nc.tensor.matmul(py[:, :],
                         st.outT[et2][:, nt * P:(nt + 1) * P],
                         wr[:, et2, :, :],
                         start=True, stop=not to_dram)
        if to_dram:
            # fold the accumulated y_acc in via an identity matmul (PE is
            # idle in the tail; saves the serial DVE adds), then evacuate on
            # alternating engines and store two n-tiles per DMA (halves the
            # serial DGE + semaphore overhead of the final stores)
            nc.tensor.matmul(py[:, :], st.ident_b[:, :], st.y_acc[nt][:, :],
                             start=False, stop=True)
            if nt % 2 == 0:
                st.ysb2 = st.norm.tile([P, 2 * D], F32, tag="y", bufs=2,
                                       name=f"y{nt}")
            _copy(nc.scalar if nt >= 4 else nc.vector,
                  st.ysb2[:, (nt % 2) * D:(nt % 2 + 1) * D], py[:, :])
            if nt % 2 == 1:
                nc.sync.dma_start(
                    st.out_d[(nt - 1) * P:(nt + 1) * P, :].rearrange(
                        "(t p) d -> p t d", p=P),
                    st.ysb2[:, :].rearrange("p (t d) -> p t d", t=2))
        elif et2 == 0:
            nc.vector.tensor_tensor(st.y_acc[nt][:, :], py[:, :], st.bo_b[:, :], Add)
        else:
            nc.vector.tensor_tensor(st.y_acc[nt][:, :], py[:, :], st.y_acc[nt][:, :], Add)
        yield


# ----------------------------------------------------------- head loop ----

def _prep_vhs(st, h, jt, sT, invS, vhs_list):
    """invS column reciprocal (DVE) + vhs tile build (Pool)."""
    nc = st.nc
    nc.vector.reciprocal(invS[:, jt:jt + 1], sT[:, jt:jt + 1])
    vt = st.head.tile([P, DH + 1], CDT, tag=f"vhs{jt}", bufs=2, name=f"vhs{h}_{jt}")
    nc.gpsimd.tensor_scalar_mul(vt[:, 0:DH], st.v[jt][:, h * DH:(h + 1) * DH],
                                invS[:, jt:jt + 1])
    nc.gpsimd.tensor_copy(vt[:, DH:DH + 1], invS[:, jt:jt + 1])
    vhs_list.append(vt)


def _avstep_mm(st, h, jt, eT, vhs_list, av_ps):
    """16 av/r matmuls for (head h, j-tile jt)."""
    nc = st.nc
    vt = vhs_list[jt]
    # start marks the whole 2KB psum bank pending-zero (lazy zeroing): only
    # the first matmul of the head's bank-group starts, only the last stops;
    # each chunk's first write then overwrites instead of accumulating.
    for it in range(8):
        nc.tensor.matmul(av_ps[:, it * DH:(it + 1) * DH],
                         eT[jt][:, it * P:(it + 1) * P],
                         vt[:, 0:DH],
                         start=(jt == 0 and it == 0), stop=(jt == 7 and it == 7))
        nc.tensor.matmul(st.r_all[:, h * 8 + it:h * 8 + it + 1],
                         eT[jt][:, it * P:(it + 1) * P],
                         vt[:, DH:DH + 1],
                         start=(jt == 0 and it == 0), stop=(jt == 7 and it == 7))


def _invr(st, h):
    iv = st.norm.tile([P, 8], F32, tag="invr", bufs=2, name=f"invr{h}")
    st.nc.vector.reciprocal(iv[:, :], st.r_all[:, h * 8:(h + 1) * 8])
    st.invr[h] = iv


def _norm_head_g(st, h, av_ps, g, use_act=False):
    """Per-partition normalize of 4 i-tiles into the pair's avn[g] tile."""
    nc = st.nc
    iv = st.invr[h]
    et2, s = h // 2, h % 2
    key = (et2, g)
    if key not in st.avn:
        st.avn[key] = st.norm.tile([P, 512], CDT, tag=f"avn{g}", bufs=2,
                                   name=f"avn{et2}_{g}")
    Copy = mybir.ActivationFunctionType.Copy
    for li in range(4):
        it = g * 4 + li
        dst = st.avn[key][:, li * P + s * DH: li * P + s * DH + DH]
        src = av_ps[:, it * DH:(it + 1) * DH]
        if use_act:
            nc.scalar.activation(dst, src, Copy, scale=iv[:, it:it + 1])
        else:
            nc.vector.tensor_scalar_mul(dst, src, iv[:, it:it + 1])


def _xbar_pair_g(st, et2, g, eng=None, tag="ps_misc"):
    # PE back-transpose (engine-local, ~0.5us latency vs ~3us for a DMA
    # XBAR hop -- this sits on the critical tail for the last pair)
    _pe_xpose_group(st, st.outT[et2], g * 512, st.avn[(et2, g)], 0,
                    f"tpo{et2}_{g}", eng=eng, tag=tag)


def _close_head(st, h, av_ps):
    """Release the av psum bank with one copy, then normalize from SBUF on
    Pool; on odd heads queue the pair's back-transposes + y-proj."""
    nc = st.nc
    avu = st.head.tile([P, 512], CDT, tag="avu", bufs=2, name=f"avu{h}")
    nc.vector.tensor_copy(avu[:, :], av_ps[:, :])
    _invr(st, h)
    et2, s = h // 2, h % 2
    for g in range(2):
        key = (et2, g)
        if key not in st.avn:
            st.avn[key] = st.norm.tile([P, 512], CDT, tag=f"avn{g}", bufs=2,
                                       name=f"avn{et2}_{g}")
        for li in range(4):
            it = g * 4 + li
            nc.gpsimd.tensor_scalar_mul(
                st.avn[key][:, li * P + s * DH: li * P + s * DH + DH],
                avu[:, it * DH:(it + 1) * DH],
                st.invr[h][:, it:it + 1])
        if h % 2 == 1:
            _xbar_pair_g(st, h // 2, g)
    if h % 2 == 1:
        st.fillers.append((f"y{h // 2}", _g_ypair(st, h // 2, h == H - 1)))


def _run(st):
    nc = st.nc
    # ---- phase 1: critical prefix ----
    # identities first (Pool compute, needed by the PE transposes)
    make_identity(nc, st.ident_b[:, :])
    # warm the PE p-state: ~3us of dependency-free dummy transposes so the
    # real prefix matmuls run at full clock (cold PE is 2x slower)
    for w in range(36):
        wps = st.ps.tile([P, P], CDT, tag="ps_sim", bufs=2, name=f"warmpe{w}")
        nc.tensor.transpose(wps[:, :], st.ident_b[:, :], st.ident_b[:, :])
    # bo (tiny) on the otherwise-unused HWDGE path
    nc.sync.dma_start(st.bo_s[:, :], st.bo_d[None, :])
    # ALL input loads as SWDGE casting DMAs (f32 DRAM -> bf16 SBUF) in
    # data-need order. One mechanism only: mixing HWDGE transposes with the
    # SWDGE stream entangles their rotating DMA-semaphore rings and
    # serializes the start.
    st.w01_sb = {}
    st.w23_sb = {}
    c0_sb = _cast_load(
        st, "c0ld", st.c_d[0:2 * P, :].rearrange("(t p) d -> p t d", p=P),
        1024, "cld0", 1)
    st.w01_sb["k"] = _cast_load(
        st, "wk01", st.w_d["k"][0:2 * P, :].rearrange("(t p) d -> p t d", p=P),
        1024, "wld", 4)
    st.w01_sb["q"] = _cast_load(
        st, "wq01", st.w_d["q"][0:2 * P, :].rearrange("(t p) d -> p t d", p=P),
        1024, "wld", 4)
    x03_sb = _cast_load(st, "x03",
                        st.x_d[0:4 * P, :].rearrange("(t p) d -> p t d", p=P),
                        2048, "xld", 2)
    x47_sb = _cast_load(st, "x47",
                        st.x_d[4 * P:8 * P, :].rearrange("(t p) d -> p t d", p=P),
                        2048, "xld", 2)
    st.c1_sb = _cast_load(
        st, "c1ld", st.c_d[2 * P:4 * P, :].rearrange("(t p) d -> p t d", p=P),
        1024, "cld1", 1)
    st.c23_sb = _cast_load(
        st, "c23ld", st.c_d[4 * P:8 * P, :].rearrange("(t p) d -> p t d", p=P),
        2048, "cld23", 1)
    st.wv_sb = _cast_load(st, "wvld",
                          st.w_d["v"][:, :].rearrange("(t p) d -> p t d", p=P),
                          2048, "wld2", 2)
    st.w23_sb["q"] = _cast_load(
        st, "wq23", st.w_d["q"][2 * P:4 * P, :].rearrange("(t p) d -> p t d", p=P),
        1024, "wld", 4)
    st.w23_sb["k"] = _cast_load(
        st, "wk23", st.w_d["k"][2 * P:4 * P, :].rearrange("(t p) d -> p t d", p=P),
        1024, "wld", 4)
    st.wo_sb = _cast_load(st, "wold",
                          st.wo_d[:, :].rearrange("(t p) d -> p t d", p=P),
                          2048, "wld2", 2)
    # critical-prefix PE transposes + first projections: alternate psum
    # tags (ps_misc / the not-yet-used ps_sim) and evacuation engines
    # (DVE / the idle-before-first-exp ACT) for a 4-deep, 2-engine pipeline
    ai = 0

    def nxt():
        nonlocal ai
        t = ("ps_misc", "ps_sim")[ai % 2]
        # ACT helps with early evacuations; the last groups gate the first
        # sim and must not sit behind ACT's 1us copies
        e = nc.scalar if (ai % 2 and ai < 8) else nc.vector
        ai += 1
        return t, e

    t, e = nxt()
    _pe_xpose_group2(st, st.cT, 0, c0_sb, 0, "tpc0", tag=t, eng=e)
    t, e = nxt()
    _pe_xpose_group2(st, st.wT["k"], 0, st.w01_sb["k"], 0, "tpwk01", tag=t, eng=e)
    t, e = nxt()
    _drain(_k_proj_chunk(st, 0, 0, tag=t, eng=e))
    t, e = nxt()
    _pe_xpose_group2(st, st.wT["q"], 0, st.w01_sb["q"], 0, "tpwq01", tag=t, eng=e)
    for g in range(2):
        t, e = nxt()
        _pe_xpose_group2(st, st.xT, g * 1024, x03_sb, g * 1024, f"tpxa{g}",
                         tag=t, eng=e)
    t, e = nxt()
    _drain(_q_proj(st, 0, 0, tag=t, eng=e))
    for g in range(2):
        t, e = nxt()
        _pe_xpose_group2(st, st.xT, 2048 + g * 1024, x47_sb, g * 1024,
                         f"tpxb{g}", tag=t, eng=e)
    t, e = nxt()
    _drain(_q_proj(st, 0, 1, tag=t, eng=e))

    st.fillers = [
        ("cstream", _g_cstream(st)),
        ("wv", _g_wv(st)),
        ("vproj", _g_vproj(st)),
        ("wrest", _g_wrest(st)),
        ("proj1", _g_proj(st, 1)),
        ("wo", _g_wo(st)),
        ("proj2", _g_proj(st, 2)),
        ("proj3", _g_proj(st, 3)),
    ]

    # ---- phase 2: head loop ----
    DEADLINES = {2: ("wrest", "proj1"), 4: ("proj2",), 6: ("proj3",)}
    pend = []  # (h, jt, eT, vhs_list, av_ps)
    for h in range(H):
        for need in DEADLINES.get(h, ()):
            _force(st, need)
        et2, ro = h // 2, (h % 2) * DH
        sT = st.head.tile([P, 8], F32, tag="sT", bufs=2, name=f"sT{h}")
        invS = st.head.tile([P, 8], F32, tag="invS", bufs=2, name=f"invS{h}")
        av_ps = st.ps.tile([P, 512], F32, tag="ps_av", bufs=1, name=f"av{h}")
        eT = []
        vhs_list = []
        for jt in range(8):
            if h == 0 and jt >= 2 and jt % 2 == 0:
                _force_until(st, "cstream", 7 * (jt // 2))
            psim = st.ps.tile([P, N], F32, tag="ps_sim", bufs=2, name=f"psim{h}_{jt}")
            for ic in range(2):
                nc.tensor.matmul(
                    psim[:, ic * 512:(ic + 1) * 512],
                    st.kT[et2][ro:ro + DH, jt * P:(jt + 1) * P],
                    st.qT[et2][ro:ro + DH, ic * 512:(ic + 1) * 512],
                    start=True, stop=True)
            e = st.head.tile([P, N], CDT, tag=f"expT{jt}", bufs=2, name=f"expT{h}_{jt}")
            if jt % 2 == 1 and jt != 7:
                # skip the 187ns ACT accumulator read on odd tiles; S[j]
                # comes from a DVE reduce over the bf16 exp tile instead
                nc.scalar.activation(e[:, :], psim[:, :], Exp, scale=SCALE)
                nc.vector.tensor_reduce(sT[:, jt:jt + 1], e[:, :],
                                        axis=mybir.AxisListType.X, op=Add)
            else:
                nc.scalar.activation(e[:, :], psim[:, :], Exp, scale=SCALE,
                                     accum_out=sT[:, jt:jt + 1])
            eT.append(e)
            pend.append((h, jt, sT, invS, eT, vhs_list, av_ps))
            lag = AV0LAG if pend[0][0] == 0 else AVLAG
            spill = 0
            while pend and (pend[0][0] < h or jt - pend[0][1] >= lag) and \
                    (spill < AVPACE or pend[0][0] == h):
                if pend[0][0] < h:
                    spill += 1
                ph, pjt, psT, pinvS, peT, pvhs, pav = pend.pop(0)
                if ph == 0:
                    _force(st, "cstream")
                    _force(st, "wv")
                    _force_until(st, "vproj", 4 * (pjt + 1))
                _prep_vhs(st, ph, pjt, psT, pinvS, pvhs)
                _avstep_mm(st, ph, pjt, peT, pvhs, pav)
                if pjt == 7:
                    _close_head(st, ph, pav)
                lag = AV0LAG if pend and pend[0][0] == 0 else AVLAG
            _budget_drain(st, FB)
    # ---- phase 3: tail ----
    while len(pend) > 1:
        ph, pjt, psT, pinvS, peT, pvhs, pav = pend.pop(0)
        _prep_vhs(st, ph, pjt, psT, pinvS, pvhs)
        _avstep_mm(st, ph, pjt, peT, pvhs, pav)
    ph, pjt, psT, pinvS, peT, pvhs, pav = pend.pop(0)
    _prep_vhs(st, ph, pjt, psT, pinvS, pvhs)
    _avstep_mm(st, ph, pjt, peT, pvhs, pav)
    # tail: ACT's chain goes FIRST through its own copy of the av psum (so
    # it is the first waiter and isn't chained behind DVE's sems), then the
    # two halves' normalize/transpose/evac pipelines run in parallel:
    # g1 on ACT from the copy, g0 on DVE straight from psum.
    nc = st.nc
    Copy = mybir.ActivationFunctionType.Copy
    avu7 = st.head.tile([P, 512], CDT, tag="avu", bufs=2, name="avu7")
    nc.scalar.copy(avu7[:, :], pav[:, :])
    iv2 = []
    for g in range(2):
        t = st.norm.tile([P, 8], F32, tag="invr", bufs=2, name=f"invr7_{g}")
        nc.vector.reciprocal(t[:, :], st.r_all[:, ph * 8:(ph + 1) * 8])
        iv2.append(t)
    st.invr[ph] = iv2[0]
    yg = _g_ypair(st, ph // 2, True)
    for g in (1, 0):
        key = (ph // 2, g)
        for li in range(4):
            it = g * 4 + li
            dst = st.avn[key][:, li * P + (ph % 2) * DH:
                              li * P + (ph % 2) * DH + DH]
            if g == 1:
                nc.scalar.activation(dst, avu7[:, it * DH:(it + 1) * DH],
                                     Copy, scale=iv2[1][:, it:it + 1])
            else:
                nc.vector.tensor_scalar_mul(
                    dst, pav[:, it * DH:(it + 1) * DH], iv2[0][:, it:it + 1])
        _xbar_pair_g(st, ph // 2, g, eng=st.nc.scalar if g == 1 else None,
                     tag="ps_sim")
    for _ in range(8):
        next(yg, None)
    _drain(yg)
    for pair in st.fillers:
        _drain(pair[1])


_CACHE = {}


def get_nc():
    if "nc" not in _CACHE:
        # Bacc (not raw Bass): its compile() runs the wait-legalization passes
        # (move_matmul_waits_to_ldweights, generate_event_semaphores) that
        # walrus codegen requires (max 1 sync wait per instruction).
        nc = bacc.Bacc("TRN2", target_bir_lowering=False, num_devices=B)
        build(nc)
        nc.compile()
        _CACHE["nc"] = nc
    return _CACHE["nc"]


def kernel(x, context, Wq, Wk, Wv, Wo, bo):
    nc = get_nc()
    w = {
        "Wq": np.ascontiguousarray(Wq, dtype=np.float32),
        "Wk": np.ascontiguousarray(Wk, dtype=np.float32),
        "Wv": np.ascontiguousarray(Wv, dtype=np.float32),
        "Wo": np.ascontiguousarray(Wo, dtype=np.float32),
        "bo": np.ascontiguousarray(bo, dtype=np.float32),
    }
    in_maps = [
        {"x": np.ascontiguousarray(x[b], dtype=np.float32),
         "context": np.ascontiguousarray(context[b], dtype=np.float32),
         **w}
        for b in range(B)
    ]
    res = run_bass_kernel_spmd(nc, in_maps, core_ids=list(range(B)))
    _CACHE["last"] = res
    return np.stack([res.results[b]["out"] for b in range(B)], axis=0)


# revision 70
# speedup vs baseline: 1.0135x; 1.0135x over previous
"""Slot-attention (softmax over queries + key renormalization) on 8 TRN2 NeuronCores.

Sharding: data-parallel over batch (b=8 -> one batch element per core, no
collectives). Per-core fused kernel, built around:

1. All inputs arrive via gpsimd SWDGE casting DMAs (f32 DRAM -> bf16 SBUF:
   half the serialized DMA-engine hold of an f32 load, no convert pass).
   All layout transposes run on the PE with a bf16 identity in [P,1024]
   single-bank bf16 psum groups (a p-state warmup keeps the PE at full
   clock through the prefix). Mixing HWDGE DMAs into this stream is
   poisonous: the rotating 8-deep DMA semaphore rings chain unrelated
   DMAs together; keep the HWDGE queue for the output stores only.

2. attn@v runs in the [i, e] orientation with the exp tiles as the
   STATIONARY operand: av[i-tile, 64] += expT[jt][:, i-slice]^T @ vhs[jt],
   full 128-row contraction (vs 64 in the [e, i] orientation) -> half the
   PE cycles. A 1-column matmul per (it, jt) accumulates the renormalizer
   r[i] into a persistent PSUM tile. Each head's av psum bank is released
   by a single copy to SBUF; normalization attn*[1/r[i]] is a
   per-partition scalar multiply on Pool from that copy, and the
   normalized [i, e] tiles are transposed back to [e, i] on the PE.

Pipeline per head h, j-tile jt: sim (PE) -> exp (ACT; S[j] via the ACT
accumulator on even tiles, via a DVE reduce on odd tiles to shorten the
serial ACT stream) -> [AVLAG steps later] invS column recip (DVE) + vhs
(Pool) + 16 av/r matmuls (PE). Prep (weight/context streams, v/q/k
projections, closed pairs' output projections) is interleaved as
budget-drained filler generators; the tail splits the last pair's
normalize/transpose/evacuate chains across DVE and ACT in parallel.
The end-to-end critical path is the serial exp stream on ACT.

Matmul operands bf16, accumulation f32 in PSUM, softmax statistics f32.
"""

import os
import sys

sys.path.insert(0, "/opt/trn_rl_repo")

import numpy as np

import concourse.bass as bass
import concourse.mybir as mybir
import concourse.tile as tile
from concourse import bacc
from concourse.bass_utils import run_bass_kernel_spmd
from concourse.masks import make_identity

B = 8
N = 1024  # queries
M = 1024  # keys
D = 512   # model dim
H = 8
DH = 64
INNER = H * DH
SCALE = DH ** -0.5
P = 128

F32 = mybir.dt.float32
CDT = mybir.dt.bfloat16

AV0LAG = int(os.environ.get("AV0LAG", "6"))
AVPACE = int(os.environ.get("AVPACE", "3"))
AVLAG = int(os.environ.get("AVLAG", "2"))
FB = int(os.environ.get("FB", "3"))
_DRAINED = {}

Exp = mybir.ActivationFunctionType.Exp
Mult = mybir.AluOpType.mult
Add = mybir.AluOpType.add


def _r3(ap, a):
    return ap.rearrange("p (a b) -> p a b", a=a)


def build(nc: bass.Bass):
    _DRAINED.clear()
    x_d = nc.declare_dram_parameter("x", [N, D], F32, isOutput=False)
    c_d = nc.declare_dram_parameter("context", [M, D], F32, isOutput=False)
    wq_d = nc.declare_dram_parameter("Wq", [INNER, D], F32, isOutput=False)
    wk_d = nc.declare_dram_parameter("Wk", [INNER, D], F32, isOutput=False)
    wv_d = nc.declare_dram_parameter("Wv", [INNER, D], F32, isOutput=False)
    wo_d = nc.declare_dram_parameter("Wo", [D, INNER], F32, isOutput=False)
    bo_d = nc.declare_dram_parameter("bo", [D], F32, isOutput=False)
    out_d = nc.declare_dram_parameter("out", [N, D], F32, isOutput=True)

    with tile.TileContext(nc) as tc:
        with tc.tile_pool(name="const", bufs=1) as const:
            ident = const.tile([P, P], F32, tag="ident")
            ident_b = const.tile([P, P], CDT, tag="ident_b")
            ones128 = const.tile([1, P], CDT, tag="ones128")
            nc.gpsimd.memset(ones128[:, :], 1.0)
            # tiny warmup exp so the ACT table load happens at t~0
            warm = const.tile([1, 2], F32, tag="warm")
            nc.gpsimd.memset(warm[:, :], 0.0)
            nc.scalar.activation(warm[:, 0:1], warm[:, 1:2], Exp, scale=1.0)
            bo_s = const.tile([1, D], F32, tag="bo_s")
            bo_sb = const.tile([1, D], CDT, tag="bo_sb")
            bo_b = const.tile([P, D], F32, tag="bo_b")

            with tc.tile_pool(name="stage", bufs=1) as stage:
                # wT[n][p, (et*4+dt)*128 + f] = W[et*128+f, dt*128+p]
                wT = {n: stage.tile([P, 16 * P], CDT, tag=f"w{n}T", name=f"w{n}T")
                      for n in ("q", "k", "v")}
                # woT[p, (dt*4+et)*128 + f] = Wo[dt*128+f, et*128+p]
                woT = stage.tile([P, 16 * P], CDT, tag="woT")
                # xT_b[p, (nt*4+dt)*128 + f] = x[nt*128+f, dt*128+p]
                xT = stage.tile([P, 32 * P], CDT, tag="xT")
                cT = stage.tile([P, 32 * P], CDT, tag="cT")
                v = [stage.tile([P, INNER], CDT, tag=f"v{t}", name=f"v{t}")
                     for t in range(8)]

                with tc.tile_pool(name="outp", bufs=1) as outp:
                    qT = [outp.tile([P, N], CDT, tag=f"qT{t}", name=f"qT{t}") for t in range(4)]
                    kT = [outp.tile([P, M], CDT, tag=f"kT{t}", name=f"kT{t}") for t in range(4)]
                    outT = [outp.tile([P, N], CDT, tag=f"outT{t}", name=f"outT{t}") for t in range(4)]
                    y_acc = [outp.tile([P, D], CDT, tag=f"y_acc{t}", name=f"y_acc{t}")
                             for t in range(8)]

                    with tc.tile_pool(name="head", bufs=1) as head, \
                         tc.tile_pool(name="norm", bufs=1) as norm, \
                         tc.tile_pool(name="ld", bufs=1) as ld:
                        ps_at = tc.alloc_tile_pool(name="ps_at", bufs=1, space="PSUM")
                        ps_rp = tc.alloc_tile_pool(name="ps_rp", bufs=1, space="PSUM")
                        r_all = ps_rp.tile([P, 64], F32, tag="r_all")
                        st = _State(nc, tc, head, norm, ld, ps_at, r_all,
                                    wT, woT, xT, cT, v, qT, kT, outT, y_acc,
                                    ident_b, ones128, bo_s, bo_sb, bo_b,
                                    x_d, c_d, wq_d, wk_d, wv_d, wo_d, bo_d, out_d)
                        st.ident = ident
                        _run(st)
                        ps_rp.release()
                        ps_at.release()
    return nc


class _State:
    def __init__(self, nc, tc, head, norm, ld, ps_at, r_all,
                 wT, woT, xT, cT, v, qT, kT, outT, y_acc,
                 ident_b, ones128, bo_s, bo_sb, bo_b,
                 x_d, c_d, wq_d, wk_d, wv_d, wo_d, bo_d, out_d):
        self.nc = nc
        self.tc = tc
        self.head = head
        self.norm = norm
        self.ld = ld
        self.ps = ps_at
        self.r_all = r_all
        self.wT = wT
        self.woT = woT
        self.xT = xT
        self.cT = cT
        self.v = v
        self.qT = qT
        self.kT = kT
        self.outT = outT
        self.y_acc = y_acc
        self.ident_b = ident_b
        self.ones128 = ones128
        self.bo_s = bo_s
        self.bo_sb = bo_sb
        self.bo_b = bo_b
        self.x_d = x_d
        self.c_d = c_d
        self.w_d = {"q": wq_d, "k": wk_d, "v": wv_d}
        self.wo_d = wo_d
        self.bo_d = bo_d
        self.out_d = out_d
        self.fillers = []
        self.invr = {}
        self.avn = {}


def _drain(g):
    if g is not None:
        for _ in g:
            pass


def _budget_drain(st, budget):
    while budget > 0 and st.fillers:
        try:
            next(st.fillers[0][1])
            _DRAINED[st.fillers[0][0]] = _DRAINED.get(st.fillers[0][0], 0) + 1
            budget -= 1
        except StopIteration:
            st.fillers.pop(0)


def _force(st, name):
    for pair in list(st.fillers):
        if pair[0] == name:
            _drain(pair[1])
            st.fillers.remove(pair)


def _force_until(st, name, count):
    while st.fillers and st.fillers[0][0] == name and \
            _DRAINED.get(name, 0) < count:
        try:
            next(st.fillers[0][1])
            _DRAINED[name] = _DRAINED.get(name, 0) + 1
        except StopIteration:
            st.fillers.pop(0)


# ---------------------------------------------------------------- prep ----

def _xbar(nc, dst3, src2):
    nc.sync.dma_start(dst3, src2, transpose=True)


def _cast_load(st, name, dram_ap, cols, tag, bufs):
    sb = st.ld.tile([P, cols], CDT, tag=tag, bufs=bufs, name=name)
    st.nc.gpsimd.dma_start(sb[:, :], dram_ap)
    return sb


def _pe_xpose_group(st, dstT, dst_off, src, src_off, name, tag="ps_misc",
                    eng=None):
    """PE-transpose four [128,128] bf16 chunks into dstT[:, dst_off:+512]."""
    nc = st.nc
    tp = st.ps.tile([P, 512], CDT, tag=tag, bufs=2, name=name)
    for i in range(4):
        nc.tensor.transpose(tp[:, i * P:(i + 1) * P],
                            src[:, src_off + i * P:src_off + (i + 1) * P],
                            st.ident_b[:, :])
    eng = eng or nc.vector
    if hasattr(eng, "tensor_copy"):
        eng.tensor_copy(dstT[:, dst_off:dst_off + 512], tp[:, :])
    else:
        eng.copy(dstT[:, dst_off:dst_off + 512], tp[:, :])


def _pe_xpose_group2(st, dstT, dst_off, src, src_off, name, tag="ps_misc",
                     eng=None):
    """PE-transpose eight [128,128] bf16 chunks (two source tiles) into
    dstT[:, dst_off:+1024] via one single-bank [P,1024] bf16 psum tile."""
    nc = st.nc
    tp = st.ps.tile([P, 1024], CDT, tag=tag, bufs=2, name=name)
    for i in range(8):
        nc.tensor.transpose(tp[:, i * P:(i + 1) * P],
                            src[:, src_off + i * P:src_off + (i + 1) * P],
                            st.ident_b[:, :])
    _copy(eng or nc.vector, dstT[:, dst_off:dst_off + 1024], tp[:, :])


def _pe_xpose_tile(st, which, nt, src, col_off, tag="ps_misc", eng=None):
    """PE-transpose one [128, 512] source tile into the blocked xT_b/cT_b."""
    dstT = st.xT if which == "x" else st.cT
    _pe_xpose_group(st, dstT, nt * 512, src, col_off, f"tp{which}{nt}",
                    tag=tag, eng=eng)


def _copy(eng, dst, src):
    if hasattr(eng, "tensor_copy"):
        eng.tensor_copy(dst, src)
    else:
        eng.copy(dst, src)


def _q_proj(st, et, ic, tag="ps_misc", eng=None):
    nc = st.nc
    pp = st.ps.tile([P, 512], F32, tag=tag, bufs=2, name=f"pq{et}_{ic}")
    xr = st.xT[:, :].rearrange("p (nt dt f) -> p dt nt f", nt=8, dt=4)
    for dt in range(4):
        nc.tensor.matmul(
            pp[:, :],
            st.wT["q"][:, (et * 4 + dt) * P:(et * 4 + dt + 1) * P],
            xr[:, dt, 4 * ic:4 * ic + 4, :],
            start=(dt == 0), stop=(dt == 3))
        yield
    _copy(eng or nc.vector, st.qT[et][:, ic * 512:(ic + 1) * 512], pp[:, :])


def _k_proj_chunk(st, et, k, tag="ps_misc", eng=None):
    nc = st.nc
    pk = st.ps.tile([P, 256], F32, tag=tag, bufs=2, name=f"pk{et}_{k}")
    cr = st.cT[:, :].rearrange("p (nt dt f) -> p dt nt f", nt=8, dt=4)
    for dt in range(4):
        nc.tensor.matmul(
            pk[:, :],
            st.wT["k"][:, (et * 4 + dt) * P:(et * 4 + dt + 1) * P],
            cr[:, dt, 2 * k:2 * k + 2, :],
            start=(dt == 0), stop=(dt == 3))
        yield
    _copy(eng or nc.vector, st.kT[et][:, k * 256:(k + 1) * 256], pk[:, :])


def _k_proj(st, et, ic):
    nc = st.nc
    pk = st.ps.tile([P, 512], F32, tag="ps_misc", bufs=2, name=f"pkf{et}_{ic}")
    cr = st.cT[:, :].rearrange("p (nt dt f) -> p dt nt f", nt=8, dt=4)
    for dt in range(4):
        nc.tensor.matmul(
            pk[:, :],
            st.wT["k"][:, (et * 4 + dt) * P:(et * 4 + dt + 1) * P],
            cr[:, dt, 4 * ic:4 * ic + 4, :],
            start=(dt == 0), stop=(dt == 3))
        yield
    nc.vector.tensor_copy(st.kT[et][:, ic * 512:(ic + 1) * 512], pk[:, :])


def _g_cstream(st):
    """Context tiles 2..7: PE-transpose + kT[0] chunks per pair."""
    for k in range(1, 4):
        src = st.c1_sb if k == 1 else st.c23_sb
        src_off = 0 if k == 1 else (k - 2) * 1024
        _pe_xpose_group2(st, st.cT, 2 * k * 512, src, src_off, f"tpc{k}")
        yield
        yield
        yield from _k_proj_chunk(st, 0, k)
        yield


def _g_wv(st):
    for g in range(2):
        _pe_xpose_group2(st, st.wT["v"], g * 1024, st.wv_sb, g * 1024,
                         f"tpwv{g}")
        yield
        yield


def _g_vproj(st):
    nc = st.nc
    wr = st.wT["v"][:, :].rearrange("p (et dt f) -> p dt et f", et=4, dt=4)
    for mt in range(8):
        pv = st.ps.tile([P, INNER], F32, tag="ps_misc", bufs=2, name=f"pv{mt}")
        for dt in range(4):
            nc.tensor.matmul(
                pv[:, :],
                st.cT[:, (mt * 4 + dt) * P:(mt * 4 + dt + 1) * P],
                wr[:, dt, :, :],
                start=(dt == 0), stop=(dt == 3))
            if dt == 3:
                nc.vector.tensor_copy(st.v[mt][:, :], pv[:, :])
            yield


def _g_wrest(st):
    """Wq/Wk e-tiles 2..3 (0..1 were transposed in the prefix)."""
    for name in ("q", "k"):
        _pe_xpose_group2(st, st.wT[name], 1024, st.w23_sb[name], 0,
                         f"tpw{name}23")
        yield
        yield
        yield


def _g_proj(st, et):
    for ic in range(2):
        yield from _q_proj(st, et, ic)
        yield
        yield from _k_proj(st, et, ic)
        yield


def _g_wo(st):
    nc = st.nc
    sb = st.wo_sb
    for g in range(2):
        _pe_xpose_group2(st, st.woT, g * 1024, sb, g * 1024, f"tpwo{g}")
        yield
        yield
    nc.vector.tensor_copy(st.bo_sb[:, :], st.bo_s[:, :])
    pbo = st.ps.tile([P, D], F32, tag="ps_misc", bufs=2, name="pbo")
    nc.tensor.matmul(pbo[:, :], st.ones128[:, :], st.bo_sb[:, :],
                     start=True, stop=True)
    nc.vector.tensor_copy(st.bo_b[:, :], pbo[:, :])
    yield


def _g_ypair(st, et2, to_dram):
    nc = st.nc
    wr = st.woT[:, :].rearrange("p (dt et f) -> p et dt f", dt=4, et=4)
    for nt in range(8):
        tag = "ps_sim" if to_dram else "ps_misc"
        py = st.ps.tile([P, D], F32, tag=tag,
                        bufs=2, name=f"py{et2}_{nt}")
        nc.tensor.matmul(py[:, :],
                         st.outT[et2][:, nt * P:(nt + 1) * P],
                         wr[:, et2, :, :],
                         start=True, stop=not to_dram)
        if to_dram:
            # fold the accumulated y_acc in via an identity matmul (PE is
            # idle in the tail; saves the serial DVE adds), then evacuate on
            # alternating engines and store
            nc.tensor.matmul(py[:, :], st.ident_b[:, :], st.y_acc[nt][:, :],
                             start=False, stop=True)
            ysb = st.norm.tile([P, D], F32, tag="y", bufs=4, name=f"y{nt}")
            _copy(nc.scalar if nt >= 4 else nc.vector, ysb[:, :], py[:, :])
            nc.sync.dma_start(st.out_d[nt * P:(nt + 1) * P, :], ysb[:, :])
        elif et2 == 0:
            nc.vector.tensor_tensor(st.y_acc[nt][:, :], py[:, :], st.bo_b[:, :], Add)
        else:
            nc.vector.tensor_tensor(st.y_acc[nt][:, :], py[:, :], st.y_acc[nt][:, :], Add)
        yield


# ----------------------------------------------------------- head loop ----

def _prep_vhs(st, h, jt, sT, invS, vhs_list):
    """invS column reciprocal (DVE) + vhs tile build (Pool)."""
    nc = st.nc
    nc.vector.reciprocal(invS[:, jt:jt + 1], sT[:, jt:jt + 1])
    vt = st.head.tile([P, DH + 1], CDT, tag=f"vhs{jt}", bufs=2, name=f"vhs{h}_{jt}")
    nc.gpsimd.tensor_scalar_mul(vt[:, 0:DH], st.v[jt][:, h * DH:(h + 1) * DH],
                                invS[:, jt:jt + 1])
    nc.gpsimd.tensor_copy(vt[:, DH:DH + 1], invS[:, jt:jt + 1])
    vhs_list.append(vt)


def _avstep_mm(st, h, jt, eT, vhs_list, av_ps):
    """16 av/r matmuls for (head h, j-tile jt)."""
    nc = st.nc
    vt = vhs_list[jt]
    # start marks the whole 2KB psum bank pending-zero (lazy zeroing): only
    # the first matmul of the head's bank-group starts, only the last stops;
    # each chunk's first write then overwrites instead of accumulating.
    for it in range(8):
        nc.tensor.matmul(av_ps[:, it * DH:(it + 1) * DH],
                         eT[jt][:, it * P:(it + 1) * P],
                         vt[:, 0:DH],
                         start=(jt == 0 and it == 0), stop=(jt == 7 and it == 7))
        nc.tensor.matmul(st.r_all[:, h * 8 + it:h * 8 + it + 1],
                         eT[jt][:, it * P:(it + 1) * P],
                         vt[:, DH:DH + 1],
                         start=(jt == 0 and it == 0), stop=(jt == 7 and it == 7))


def _invr(st, h):
    iv = st.norm.tile([P, 8], F32, tag="invr", bufs=2, name=f"invr{h}")
    st.nc.vector.reciprocal(iv[:, :], st.r_all[:, h * 8:(h + 1) * 8])
    st.invr[h] = iv


def _norm_head_g(st, h, av_ps, g, use_act=False):
    """Per-partition normalize of 4 i-tiles into the pair's avn[g] tile."""
    nc = st.nc
    iv = st.invr[h]
    et2, s = h // 2, h % 2
    key = (et2, g)
    if key not in st.avn:
        st.avn[key] = st.norm.tile([P, 512], CDT, tag=f"avn{g}", bufs=2,
                                   name=f"avn{et2}_{g}")
    Copy = mybir.ActivationFunctionType.Copy
    for li in range(4):
        it = g * 4 + li
        dst = st.avn[key][:, li * P + s * DH: li * P + s * DH + DH]
        src = av_ps[:, it * DH:(it + 1) * DH]
        if use_act:
            nc.scalar.activation(dst, src, Copy, scale=iv[:, it:it + 1])
        else:
            nc.vector.tensor_scalar_mul(dst, src, iv[:, it:it + 1])


def _xbar_pair_g(st, et2, g, eng=None, tag="ps_misc"):
    # PE back-transpose (engine-local, ~0.5us latency vs ~3us for a DMA
    # XBAR hop -- this sits on the critical tail for the last pair)
    _pe_xpose_group(st, st.outT[et2], g * 512, st.avn[(et2, g)], 0,
                    f"tpo{et2}_{g}", eng=eng, tag=tag)


def _close_head(st, h, av_ps):
    """Release the av psum bank with one copy, then normalize from SBUF on
    Pool; on odd heads queue the pair's back-transposes + y-proj."""
    nc = st.nc
    avu = st.head.tile([P, 512], CDT, tag="avu", bufs=2, name=f"avu{h}")
    nc.vector.tensor_copy(avu[:, :], av_ps[:, :])
    _invr(st, h)
    et2, s = h // 2, h % 2
    for g in range(2):
        key = (et2, g)
        if key not in st.avn:
            st.avn[key] = st.norm.tile([P, 512], CDT, tag=f"avn{g}", bufs=2,
                                       name=f"avn{et2}_{g}")
        for li in range(4):
            it = g * 4 + li
            nc.gpsimd.tensor_scalar_mul(
                st.avn[key][:, li * P + s * DH: li * P + s * DH + DH],
                avu[:, it * DH:(it + 1) * DH],
                st.invr[h][:, it:it + 1])
        if h % 2 == 1:
            _xbar_pair_g(st, h // 2, g)
    if h % 2 == 1:
        st.fillers.append((f"y{h // 2}", _g_ypair(st, h // 2, h == H - 1)))


def _run(st):
    nc = st.nc
    # ---- phase 1: critical prefix ----
    # identities first (Pool compute, needed by the PE transposes)
    make_identity(nc, st.ident_b[:, :])
    # warm the PE p-state: ~3us of dependency-free dummy transposes so the
    # real prefix matmuls run at full clock (cold PE is 2x slower)
    for w in range(36):
        wps = st.ps.tile([P, P], CDT, tag="ps_sim", bufs=2, name=f"warmpe{w}")
        nc.tensor.transpose(wps[:, :], st.ident_b[:, :], st.ident_b[:, :])
    # bo (tiny) on the otherwise-unused HWDGE path
    nc.sync.dma_start(st.bo_s[:, :], st.bo_d[None, :])
    # ALL input loads as SWDGE casting DMAs (f32 DRAM -> bf16 SBUF) in
    # data-need order. One mechanism only: mixing HWDGE transposes with the
    # SWDGE stream entangles their rotating DMA-semaphore rings and
    # serializes the start.
    st.w01_sb = {}
    st.w23_sb = {}
    c0_sb = _cast_load(
        st, "c0ld", st.c_d[0:2 * P, :].rearrange("(t p) d -> p t d", p=P),
        1024, "cld0", 1)
    st.w01_sb["k"] = _cast_load(
        st, "wk01", st.w_d["k"][0:2 * P, :].rearrange("(t p) d -> p t d", p=P),
        1024, "wld", 4)
    st.w01_sb["q"] = _cast_load(
        st, "wq01", st.w_d["q"][0:2 * P, :].rearrange("(t p) d -> p t d", p=P),
        1024, "wld", 4)
    x03_sb = _cast_load(st, "x03",
                        st.x_d[0:4 * P, :].rearrange("(t p) d -> p t d", p=P),
                        2048, "xld", 2)
    x47_sb = _cast_load(st, "x47",
                        st.x_d[4 * P:8 * P, :].rearrange("(t p) d -> p t d", p=P),
                        2048, "xld", 2)
    st.c1_sb = _cast_load(
        st, "c1ld", st.c_d[2 * P:4 * P, :].rearrange("(t p) d -> p t d", p=P),
        1024, "cld1", 1)
    st.c23_sb = _cast_load(
        st, "c23ld", st.c_d[4 * P:8 * P, :].rearrange("(t p) d -> p t d", p=P),
        2048, "cld23", 1)
    st.wv_sb = _cast_load(st, "wvld",
                          st.w_d["v"][:, :].rearrange("(t p) d -> p t d", p=P),
                          2048, "wld2", 2)
    st.w23_sb["q"] = _cast_load(
        st, "wq23", st.w_d["q"][2 * P:4 * P, :].rearrange("(t p) d -> p t d", p=P),
        1024, "wld", 4)
    st.w23_sb["k"] = _cast_load(
        st, "wk23", st.w_d["k"][2 * P:4 * P, :].rearrange("(t p) d -> p t d", p=P),
        1024, "wld", 4)
    st.wo_sb = _cast_load(st, "wold",
                          st.wo_d[:, :].rearrange("(t p) d -> p t d", p=P),
                          2048, "wld2", 2)
    # critical-prefix PE transposes + first projections: alternate psum
    # tags (ps_misc / the not-yet-used ps_sim) and evacuation engines
    # (DVE / the idle-before-first-exp ACT) for a 4-deep, 2-engine pipeline
    ai = 0

    def nxt():
        nonlocal ai
        t = ("ps_misc", "ps_sim")[ai % 2]
        # ACT helps with early evacuations; the last groups gate the first
        # sim and must not sit behind ACT's 1us copies
        e = nc.scalar if (ai % 2 and ai < 8) else nc.vector
        ai += 1
        return t, e

    t, e = nxt()
    _pe_xpose_group2(st, st.cT, 0, c0_sb, 0, "tpc0", tag=t, eng=e)
    t, e = nxt()
    _pe_xpose_group2(st, st.wT["k"], 0, st.w01_sb["k"], 0, "tpwk01", tag=t, eng=e)
    t, e = nxt()
    _drain(_k_proj_chunk(st, 0, 0, tag=t, eng=e))
    t, e = nxt()
    _pe_xpose_group2(st, st.wT["q"], 0, st.w01_sb["q"], 0, "tpwq01", tag=t, eng=e)
    for g in range(2):
        t, e = nxt()
        _pe_xpose_group2(st, st.xT, g * 1024, x03_sb, g * 1024, f"tpxa{g}",
                         tag=t, eng=e)
    t, e = nxt()
    _drain(_q_proj(st, 0, 0, tag=t, eng=e))
    for g in range(2):
        t, e = nxt()
        _pe_xpose_group2(st, st.xT, 2048 + g * 1024, x47_sb, g * 1024,
                         f"tpxb{g}", tag=t, eng=e)
    t, e = nxt()
    _drain(_q_proj(st, 0, 1, tag=t, eng=e))

    st.fillers = [
        ("cstream", _g_cstream(st)),
        ("wv", _g_wv(st)),
        ("vproj", _g_vproj(st)),
        ("wrest", _g_wrest(st)),
        ("proj1", _g_proj(st, 1)),
        ("wo", _g_wo(st)),
        ("proj2", _g_proj(st, 2)),
        ("proj3", _g_proj(st, 3)),
    ]

    # ---- phase 2: head loop ----
    DEADLINES = {2: ("wrest", "proj1"), 4: ("proj2",), 6: ("proj3",)}
    pend = []  # (h, jt, eT, vhs_list, av_ps)
    for h in range(H):
        for need in DEADLINES.get(h, ()):
            _force(st, need)
        et2, ro = h // 2, (h % 2) * DH
        sT = st.head.tile([P, 8], F32, tag="sT", bufs=2, name=f"sT{h}")
        invS = st.head.tile([P, 8], F32, tag="invS", bufs=2, name=f"invS{h}")
        av_ps = st.ps.tile([P, 512], F32, tag="ps_av", bufs=1, name=f"av{h}")
        eT = []
        vhs_list = []
        for jt in range(8):
            if h == 0 and jt >= 2 and jt % 2 == 0:
                _force_until(st, "cstream", 7 * (jt // 2))
            psim = st.ps.tile([P, N], F32, tag="ps_sim", bufs=2, name=f"psim{h}_{jt}")
            for ic in range(2):
                nc.tensor.matmul(
                    psim[:, ic * 512:(ic + 1) * 512],
                    st.kT[et2][ro:ro + DH, jt * P:(jt + 1) * P],
                    st.qT[et2][ro:ro + DH, ic * 512:(ic + 1) * 512],
                    start=True, stop=True)
            e = st.head.tile([P, N], CDT, tag=f"expT{jt}", bufs=2, name=f"expT{h}_{jt}")
            if jt % 2 == 1 and jt != 7:
                # skip the 187ns ACT accumulator read on odd tiles; S[j]
                # comes from a DVE reduce over the bf16 exp tile instead
                nc.scalar.activation(e[:, :], psim[:, :], Exp, scale=SCALE)
                nc.vector.tensor_reduce(sT[:, jt:jt + 1], e[:, :],
                                        axis=mybir.AxisListType.X, op=Add)
            else:
                nc.scalar.activation(e[:, :], psim[:, :], Exp, scale=SCALE,
                                     accum_out=sT[:, jt:jt + 1])
            eT.append(e)
            pend.append((h, jt, sT, invS, eT, vhs_list, av_ps))
            lag = AV0LAG if pend[0][0] == 0 else AVLAG
            spill = 0
            while pend and (pend[0][0] < h or jt - pend[0][1] >= lag) and \
                    (spill < AVPACE or pend[0][0] == h):
                if pend[0][0] < h:
                    spill += 1
                ph, pjt, psT, pinvS, peT, pvhs, pav = pend.pop(0)
                if ph == 0:
                    _force(st, "cstream")
                    _force(st, "wv")
                    _force_until(st, "vproj", 4 * (pjt + 1))
                _prep_vhs(st, ph, pjt, psT, pinvS, pvhs)
                _avstep_mm(st, ph, pjt, peT, pvhs, pav)
                if pjt == 7:
                    _close_head(st, ph, pav)
                lag = AV0LAG if pend and pend[0][0] == 0 else AVLAG
            _budget_drain(st, FB)
    # ---- phase 3: tail ----
    while len(pend) > 1:
        ph, pjt, psT, pinvS, peT, pvhs, pav = pend.pop(0)
        _prep_vhs(st, ph, pjt, psT, pinvS, pvhs)
        _avstep_mm(st, ph, pjt, peT, pvhs, pav)
    ph, pjt, psT, pinvS, peT, pvhs, pav = pend.pop(0)
    _prep_vhs(st, ph, pjt, psT, pinvS, pvhs)
    _avstep_mm(st, ph, pjt, peT, pvhs, pav)
    # tail: ACT's chain goes FIRST through its own copy of the av psum (so
    # it is the first waiter and isn't chained behind DVE's sems), then the
    # two halves' normalize/transpose/evac pipelines run in parallel:
    # g1 on ACT from the copy, g0 on DVE straight from psum.
    nc = st.nc
    Copy = mybir.ActivationFunctionType.Copy
    avu7 = st.head.tile([P, 512], CDT, tag="avu", bufs=2, name="avu7")
    nc.scalar.copy(avu7[:, :], pav[:, :])
    iv2 = []
    for g in range(2):
        t = st.norm.tile([P, 8], F32, tag="invr", bufs=2, name=f"invr7_{g}")
        nc.vector.reciprocal(t[:, :], st.r_all[:, ph * 8:(ph + 1) * 8])
        iv2.append(t)
    st.invr[ph] = iv2[0]
    yg = _g_ypair(st, ph // 2, True)
    for g in (1, 0):
        key = (ph // 2, g)
        for li in range(4):
            it = g * 4 + li
            dst = st.avn[key][:, li * P + (ph % 2) * DH:
                              li * P + (ph % 2) * DH + DH]
            if g == 1:
                nc.scalar.activation(dst, avu7[:, it * DH:(it + 1) * DH],
                                     Copy, scale=iv2[1][:, it:it + 1])
            else:
                nc.vector.tensor_scalar_mul(
                    dst, pav[:, it * DH:(it + 1) * DH], iv2[0][:, it:it + 1])
        _xbar_pair_g(st, ph // 2, g, eng=st.nc.scalar if g == 1 else None,
                     tag="ps_sim")
    for _ in range(8):
        next(yg, None)
    _drain(yg)
    for pair in st.fillers:
        _drain(pair[1])


_CACHE = {}


def get_nc():
    if "nc" not in _CACHE:
        # Bacc (not raw Bass): its compile() runs the wait-legalization passes
        # (move_matmul_waits_to_ldweights, generate_event_semaphores) that
        # walrus codegen requires (max 1 sync wait per instruction).
        nc = bacc.Bacc("TRN2", target_bir_lowering=False, num_devices=B)
        build(nc)
        nc.compile()
        _CACHE["nc"] = nc
    return _CACHE["nc"]


def kernel(x, context, Wq, Wk, Wv, Wo, bo):
    nc = get_nc()
    w = {
        "Wq": np.ascontiguousarray(Wq, dtype=np.float32),
        "Wk": np.ascontiguousarray(Wk, dtype=np.float32),
        "Wv": np.ascontiguousarray(Wv, dtype=np.float32),
        "Wo": np.ascontiguousarray(Wo, dtype=np.float32),
        "bo": np.ascontiguousarray(bo, dtype=np.float32),
    }
    in_maps = [
        {"x": np.ascontiguousarray(x[b], dtype=np.float32),
         "context": np.ascontiguousarray(context[b], dtype=np.float32),
         **w}
        for b in range(B)
    ]
    res = run_bass_kernel_spmd(nc, in_maps, core_ids=list(range(B)))
    _CACHE["last"] = res
    return np.stack([res.results[b]["out"] for b in range(B)], axis=0)
